# revision 11
# baseline (speedup 1.0000x reference)
"""MaxAttention Trainium2 Bass kernel.

Problem (hardcoded shapes): x (2, 1024, 768), W_qkv (4, 2304, 768),
W_out (768, 768), b_out (768,), heads=12.

reference:
  qkv = einsum('bnd,rcd->bnrc', x, W_qkv); q,k,v = split(qkv, 3, -1)
  per (b, h, r): dots = q @ k.T * dh**-0.5 ; attn_r = softmax(dots)
  attn = max_r attn_r ; out = attn @ v[route 0] ; out @ W_out.T + b_out

Sharding: 8 cores = 2 batches x 4 head-groups (3 heads each). Each core
computes its batch's QKV slices and attention for its 3 heads. Per head, an
AllToAll over the batch's 4-core group redistributes attention outputs from
head-major to sequence-major; each core then computes the full output
projection for its disjoint 256-row slice (host does a pure concatenation).
The first two heads' AllToAlls hide under the next head's compute.

Per-core inputs (host-prepped, bf16):
  xT    [768, 1024]   x[b].T
  wqkT  [768, 1536]   q cols (r,j) blocks of 64, then k cols
  wvT   [768, 192]    route-0 v weights for the core's heads
  woT   [3, 256, 768] per-head-slot W_out rows, (src-rank, dh) ordered
  biasb [128, 768]    b_out broadcast (f32)
"""

import numpy as np
import ml_dtypes
from contextlib import ExitStack

import concourse.bass as bass
import concourse.tile as tile
from concourse import bacc, mybir, bass_utils

B, N, D = 2, 1024, 768
ROUTES, INNER, HEADS = 4, 768, 12
DH = 64
HPC = 3                      # heads per core
GROUP = 4                    # cores per batch
NCORES = 8
NT = N // 128                # 8 n-tiles
KD = D // 128                # 6 d-tiles
CQK = 2 * ROUTES * HPC * DH  # 1536: q then k columns per core
NS = N // GROUP              # 256 output rows per core
BF16 = mybir.dt.bfloat16
F32 = mybir.dt.float32

_CACHE = {}


def _build_nc():
    nc = bacc.Bacc("TRN2", target_bir_lowering=False, debug=False,
                   num_devices=NCORES)
    Exp = mybir.ActivationFunctionType.Exp

    xT_d = nc.dram_tensor("xT", [D, N], BF16, kind="ExternalInput").ap()
    wqkT_d = nc.dram_tensor("wqkT", [D, CQK], BF16, kind="ExternalInput").ap()
    wvT_d = nc.dram_tensor("wvT", [D, HPC * DH], BF16, kind="ExternalInput").ap()
    woT_d = nc.dram_tensor("woT", [HPC, NCORES * DH, D], BF16,
                           kind="ExternalInput").ap()
    bias_d = nc.dram_tensor("biasb", [128, D], F32, kind="ExternalInput").ap()
    out_d = nc.dram_tensor("out", [NS, D], F32, kind="ExternalOutput").ap()

    a2ain = [nc.dram_tensor(f"a2ain{j}", [NCORES * DH, NS], BF16).ap()
             for j in range(HPC)]
    a2aout = [nc.dram_tensor(f"a2aout{j}", [NCORES * DH, NS], BF16).ap()
              for j in range(HPC)]
    RG = [[0, 1, 2, 3, 4, 5, 6, 7]]

    with tile.TileContext(nc) as tc, ExitStack() as ctx:
        def pool(name, bufs, **kw):
            return ctx.enter_context(tc.tile_pool(name=name, bufs=bufs, **kw))

        const = pool("const", 1)
        psumA = pool("psumA", 2, space="PSUM")  # dots: [128,1024] f32, 2 banks x2
        psumB = pool("psumB", 2, space="PSUM")  # v/av/qk2/proj: 2 banks x2
        attp = pool("attp", 2)
        accp = pool("accp", 3)
        ep = pool("ep", 6)
        sp = pool("sp", 3)
        pop = pool("pop", 3)

        # ---- input loads
        xT = [const.tile([128, N], BF16, tag=f"xT{k}", name=f"xT{k}")
              for k in range(KD)]
        wqkT = [const.tile([128, CQK], BF16, tag=f"wqkT{k}", name=f"wqkT{k}")
                for k in range(KD)]
        wvT = [const.tile([128, HPC * DH], BF16, tag=f"wvT{k}", name=f"wvT{k}")
               for k in range(KD)]
        for k in range(KD):
            nc.sync.dma_start(xT[k][:], xT_d[k * 128:(k + 1) * 128, :])
            nc.sync.dma_start(wqkT[k][:], wqkT_d[k * 128:(k + 1) * 128, :])
            nc.sync.dma_start(wvT[k][:], wvT_d[k * 128:(k + 1) * 128, :])
        # woT[j]: two K-tiles of [128, 768] each
        woT = [[const.tile([128, D], BF16, tag=f"woT{j}{kt}", name=f"woT{j}{kt}")
                for kt in range(4)] for j in range(HPC)]
        for j in range(HPC):
            for kt in range(4):
                nc.sync.dma_start(woT[j][kt][:],
                                  woT_d[j, kt * 128:(kt + 1) * 128, :])
        bias_s = const.tile([128, D], F32, tag="bias")
        nc.sync.dma_start(bias_s[:], bias_d[:, :])

        # ---- qkT[c, n] c-tiles (6 q + 6 k); emit the 8 tiles head 0 needs
        # first so its dots can start early.
        qkT = [const.tile([128, N], BF16, tag=f"qkT{t}", name=f"qkT{t}")
               for t in range(12)]

        def emit_qkT(mc):
            ps = psumA.tile([128, N], F32, tag="ps", name="qk_ps")
            for k in range(KD):
                for h in range(2):
                    nc.tensor.matmul(ps[:, h * 512:(h + 1) * 512],
                                     wqkT[k][:, mc * 128:(mc + 1) * 128],
                                     xT[k][:, h * 512:(h + 1) * 512],
                                     start=(k == 0), stop=(k == KD - 1))
            nc.vector.tensor_copy(qkT[mc][:], ps[:])

        def emit_qkT_late(mc):
            ps = psumB.tile([128, N], F32, tag="pb", name="qk_ps2")
            for k in range(KD):
                for h in range(2):
                    nc.tensor.matmul(ps[:, h * 512:(h + 1) * 512],
                                     wqkT[k][:, mc * 128:(mc + 1) * 128],
                                     xT[k][:, h * 512:(h + 1) * 512],
                                     start=(k == 0), stop=(k == KD - 1))
            nc.vector.tensor_copy(qkT[mc][:], ps[:])

        for mc in (0, 6, 1, 7, 3, 9, 4, 10):  # j=0 routes: q {0,1,3,4}, k +6
            emit_qkT(mc)

        # ---- v[n, 192] + late qkT (fill PE slack under head-0's ACT-bound
        # dots phase)
        v_s = [const.tile([128, HPC * DH], BF16, tag=f"v{m}", name=f"v{m}")
               for m in range(NT)]

        def emit_v(mt):
            ps = psumB.tile([128, HPC * DH], F32, tag="pb", name="v_ps")
            for k in range(KD):
                nc.tensor.matmul(ps[:], xT[k][:, mt * 128:(mt + 1) * 128],
                                 wvT[k][:], start=(k == 0), stop=(k == KD - 1))
            nc.vector.tensor_copy(v_s[mt][:], ps[:])

        # ---- attention
        aoT = [const.tile([64, N], BF16, tag=f"aoT{j}", name=f"aoT{j}")
               for j in range(HPC)]

        def emit_av(j, attnT):
            av = psumB.tile([64, N], F32, tag="pb", name="av_ps")
            attnT4 = attnT[:].rearrange("p (a m q) -> p a m q", a=NT, m=NT)
            for mt in range(NT):
                for h in range(2):
                    nc.tensor.matmul(av[:, h * 512:(h + 1) * 512],
                                     v_s[mt][:, j * DH:(j + 1) * DH],
                                     attnT4[:, h * 4:(h + 1) * 4, mt, :],
                                     start=(mt == 0), stop=(mt == NT - 1))
            nc.vector.tensor_copy(aoT[j][:], av[:])
            # scatter aoT columns per destination rank: shard for global
            # rank s carries the columns of group-position s % GROUP (the
            # other batch's cores ignore it via zeroed woT rows)
            for dhalf in range(2):
                for s in range(GROUP):
                    r0 = dhalf * GROUP * DH + s * DH
                    nc.sync.dma_start(
                        a2ain[j][r0:r0 + DH, :],
                        aoT[j][:, s * NS:(s + 1) * NS])
            nc.gpsimd.collective_compute(
                "AllToAll", mybir.AluOpType.bypass, replica_groups=RG,
                ins=[a2ain[j][:, :].opt()], outs=[a2aout[j][:, :].opt()])

        attnT_prev = None
        for j in range(HPC):
            attnT = attp.tile([128, NT * N], BF16, tag="attnT", name="attnT")
            for qt in range(NT):
                acc = accp.tile([128, N], BF16, tag="acc", name="acc")
                s4 = sp.tile([128, ROUTES], F32, tag="s4", name="s4")
                r4 = sp.tile([128, ROUTES], F32, tag="r4", name="r4")
                es = []
                for r in range(ROUTES):
                    rj = r * HPC + j
                    tq, oq = divmod(rj * DH, 128)
                    dots = psumA.tile([128, N], F32, tag="ps", name="dots")
                    for h in range(2):
                        nc.tensor.matmul(
                            dots[:, h * 512:(h + 1) * 512],
                            qkT[tq][oq:oq + DH, qt * 128:(qt + 1) * 128],
                            qkT[6 + tq][oq:oq + DH, h * 512:(h + 1) * 512],
                            start=True, stop=True)
                    e = ep.tile([128, N], BF16, tag="e", name="e")
                    nc.scalar.activation(e[:], dots[:], Exp, scale=DH ** -0.5,
                                         accum_out=s4[:, r:r + 1])
                    es.append(e)
                nc.vector.reciprocal(r4[:], s4[:])
                nc.vector.tensor_scalar(acc[:], es[0][:], r4[:, 0:1], None,
                                        op0=mybir.AluOpType.mult)
                for r in range(1, ROUTES):
                    nc.vector.scalar_tensor_tensor(
                        acc[:], es[r][:], r4[:, r:r + 1], acc[:],
                        op0=mybir.AluOpType.mult, op1=mybir.AluOpType.max)
                # transpose acc [nq, m] -> attnT qt-block [m-local, mt, nq]
                dst = attnT[:, qt * N:(qt + 1) * N].rearrange(
                    "p (m q) -> p m q", m=NT)
                nc.sync.dma_start_transpose(dst, acc[:])
                # fill PE slack during head 0 with v and the late qkT tiles;
                # run the previous head's attn@v during heads 1 and 2
                if j == 0:
                    if qt < 4:
                        emit_qkT_late((2, 8, 5, 11)[qt])
                    else:
                        emit_v(qt - 4)
                        emit_v(qt)
                elif qt == 2:
                    emit_av(j - 1, attnT_prev)
            attnT_prev = attnT
        emit_av(HPC - 1, attnT_prev)

        # ---- projection for this core's 256 rows, over all 12 heads
        a2a_sb = [[const.tile([128, NS], BF16, tag=f"a2a{j}{kt}",
                              name=f"a2a{j}{kt}") for kt in range(4)]
                  for j in range(HPC)]
        for j in range(HPC):
            for kt in range(4):
                nc.sync.dma_start(a2a_sb[j][kt][:],
                                  a2aout[j][kt * 128:(kt + 1) * 128, :])
        JK = [(j, kt) for j in range(HPC) for kt in range(4)]
        for mchunk in range(2):
            ps = psumB.tile([128, D], F32, tag="pb", name="proj_ps")
            for c0, c1 in ((0, 512), (512, 768)):
                for i, (j, kt) in enumerate(JK):
                    nc.tensor.matmul(
                        ps[:, c0:c1],
                        a2a_sb[j][kt][:, mchunk * 128:(mchunk + 1) * 128],
                        woT[j][kt][:, c0:c1],
                        start=(i == 0), stop=(i == len(JK) - 1))
            po = pop.tile([128, D], F32, tag="po", name="po")
            nc.vector.tensor_add(po[:], ps[:], bias_s[:])
            nc.sync.dma_start(out_d[mchunk * 128:(mchunk + 1) * 128, :], po[:])

    nc.compile()
    return nc


def _prep_core_inputs(x, W_qkv, W_out, b_out, c):
    b, g = divmod(c, GROUP)
    bf = ml_dtypes.bfloat16
    xT = np.ascontiguousarray(x[b].T).astype(bf)
    wqkT = np.empty((D, CQK), dtype=bf)
    for r in range(ROUTES):
        for j in range(HPC):
            h = HPC * g + j
            rj = r * HPC + j
            wqkT[:, rj * DH:(rj + 1) * DH] = \
                W_qkv[r, h * DH:(h + 1) * DH, :].T.astype(bf)
            wqkT[:, INNER + rj * DH:INNER + (rj + 1) * DH] = \
                W_qkv[r, INNER + h * DH:INNER + (h + 1) * DH, :].T.astype(bf)
    wvT = np.empty((D, HPC * DH), dtype=bf)
    for j in range(HPC):
        h = HPC * g + j
        wvT[:, j * DH:(j + 1) * DH] = \
            W_qkv[0, 2 * INNER + h * DH:2 * INNER + (h + 1) * DH, :].T.astype(bf)
    # After the 8-rank per-head AllToAll, a2aout rows are (global source
    # rank s, dh t). Rows from the other batch's cores get zero weights.
    woT = np.zeros((HPC, NCORES * DH, D), dtype=bf)
    for j in range(HPC):
        for s in range(GROUP):
            sg = b * GROUP + s
            h = HPC * s + j
            woT[j, sg * DH:(sg + 1) * DH, :] = \
                W_out[:, h * DH:(h + 1) * DH].T.astype(bf)
    biasb = np.ascontiguousarray(
        np.broadcast_to(b_out.astype(np.float32), (128, D)))
    return {"xT": xT, "wqkT": wqkT, "wvT": wvT, "woT": woT, "biasb": biasb}


def _run(in_maps, trace=False, tmpdir=None):
    if "nc" not in _CACHE:
        _CACHE["nc"] = _build_nc()
    return bass_utils.run_bass_kernel_spmd(
        _CACHE["nc"], in_maps, core_ids=list(range(NCORES)),
        trace=trace, tmpdir=tmpdir)


def kernel(x, W_qkv, W_out, b_out, heads, _trace=False, _tmpdir=None):
    x = np.asarray(x, dtype=np.float32)
    W_qkv = np.asarray(W_qkv, dtype=np.float32)
    W_out = np.asarray(W_out, dtype=np.float32)
    b_out = np.asarray(b_out, dtype=np.float32)
    assert int(heads) == HEADS
    in_maps = [_prep_core_inputs(x, W_qkv, W_out, b_out, c)
               for c in range(NCORES)]
    res = _run(in_maps, trace=_trace, tmpdir=_tmpdir)
    out = np.empty((B, N, D), dtype=np.float32)
    for c in range(NCORES):
        b, g = divmod(c, GROUP)
        out[b, g * 256:(g + 1) * 256, :] = res.results[c]["out"]
    if _trace:
        return out, res
    return out


# revision 12
# speedup vs baseline: 1.0497x; 1.0497x over previous
"""MaxAttention Trainium2 Bass kernel.

Problem (hardcoded shapes): x (2, 1024, 768), W_qkv (4, 2304, 768),
W_out (768, 768), b_out (768,), heads=12.

reference:
  qkv = einsum('bnd,rcd->bnrc', x, W_qkv); q,k,v = split(qkv, 3, -1)
  per (b, h, r): dots = q @ k.T * dh**-0.5 ; attn_r = softmax(dots)
  attn = max_r attn_r ; out = attn @ v[route 0] ; out @ W_out.T + b_out

Sharding: 8 cores = 2 batches x 4 head-groups (3 heads each). Each core
computes its batch's QKV slices and attention for its 3 heads. Per head, an
AllToAll over the batch's 4-core group redistributes attention outputs from
head-major to sequence-major; each core then computes the full output
projection for its disjoint 256-row slice (host does a pure concatenation).
The first two heads' AllToAlls hide under the next head's compute.

Per-core inputs (host-prepped, bf16):
  xT    [768, 1024]   x[b].T
  wqkT  [768, 1536]   q cols (r,j) blocks of 64, then k cols
  wvT   [768, 192]    route-0 v weights for the core's heads
  woT   [3, 256, 768] per-head-slot W_out rows, (src-rank, dh) ordered
  biasb [128, 768]    b_out broadcast (f32)
"""

import numpy as np
import ml_dtypes
from contextlib import ExitStack

import concourse.bass as bass
import concourse.tile as tile
from concourse import bacc, mybir, bass_utils

B, N, D = 2, 1024, 768
ROUTES, INNER, HEADS = 4, 768, 12
DH = 64
HPC = 3                      # heads per core
GROUP = 4                    # cores per batch
NCORES = 8
NT = N // 128                # 8 n-tiles
KD = D // 128                # 6 d-tiles
CQK = 2 * ROUTES * HPC * DH  # 1536: q then k columns per core
NS = N // GROUP              # 256 output rows per core
BF16 = mybir.dt.bfloat16
F32 = mybir.dt.float32

_CACHE = {}


def _build_nc():
    nc = bacc.Bacc("TRN2", target_bir_lowering=False, debug=False,
                   num_devices=NCORES)
    Exp = mybir.ActivationFunctionType.Exp

    xT_d = nc.dram_tensor("xT", [D, N], BF16, kind="ExternalInput").ap()
    wqkT_d = nc.dram_tensor("wqkT", [D, CQK], BF16, kind="ExternalInput").ap()
    wvT_d = nc.dram_tensor("wvT", [D, HPC * DH], BF16, kind="ExternalInput").ap()
    woT_d = nc.dram_tensor("woT", [HPC, NCORES * DH, D], BF16,
                           kind="ExternalInput").ap()
    bias_d = nc.dram_tensor("biasb", [128, D], F32, kind="ExternalInput").ap()
    out_d = nc.dram_tensor("out", [NS, D], F32, kind="ExternalOutput").ap()

    a2ain = [nc.dram_tensor(f"a2ain{j}", [NCORES * DH, NS], BF16).ap()
             for j in range(HPC)]
    a2aout = [nc.dram_tensor(f"a2aout{j}", [NCORES * DH, NS], BF16).ap()
              for j in range(HPC)]
    RG = [[0, 1, 2, 3, 4, 5, 6, 7]]

    with tile.TileContext(nc) as tc, ExitStack() as ctx:
        def pool(name, bufs, **kw):
            return ctx.enter_context(tc.tile_pool(name=name, bufs=bufs, **kw))

        const = pool("const", 1)
        psumA = pool("psumA", 2, space="PSUM")  # dots: [128,1024] f32, 2 banks x2
        psumB = pool("psumB", 2, space="PSUM")  # v/av/qk2/proj: 2 banks x2
        attp = pool("attp", 2)
        accp = pool("accp", 3)
        ep = pool("ep", 6)
        sp = pool("sp", 3)
        pop = pool("pop", 3)

        # ---- input loads
        xT = [const.tile([128, N], BF16, tag=f"xT{k}", name=f"xT{k}")
              for k in range(KD)]
        wqkT = [const.tile([128, CQK], BF16, tag=f"wqkT{k}", name=f"wqkT{k}")
                for k in range(KD)]
        wvT = [const.tile([128, HPC * DH], BF16, tag=f"wvT{k}", name=f"wvT{k}")
               for k in range(KD)]
        for k in range(KD):
            nc.sync.dma_start(xT[k][:], xT_d[k * 128:(k + 1) * 128, :])
            nc.sync.dma_start(wqkT[k][:], wqkT_d[k * 128:(k + 1) * 128, :])
            nc.sync.dma_start(wvT[k][:], wvT_d[k * 128:(k + 1) * 128, :])
        # woT[j]: two K-tiles of [128, 768] each
        woT = [[const.tile([128, D], BF16, tag=f"woT{j}{kt}", name=f"woT{j}{kt}")
                for kt in range(4)] for j in range(HPC)]
        for j in range(HPC):
            for kt in range(4):
                nc.sync.dma_start(woT[j][kt][:],
                                  woT_d[j, kt * 128:(kt + 1) * 128, :])
        bias_s = const.tile([128, D], F32, tag="bias")
        nc.sync.dma_start(bias_s[:], bias_d[:, :])

        # ---- qkT[c, n] c-tiles (6 q + 6 k); emit the 8 tiles head 0 needs
        # first so its dots can start early.
        qkT = [const.tile([128, N], BF16, tag=f"qkT{t}", name=f"qkT{t}")
               for t in range(12)]

        def emit_qkT(mc):
            ps = psumA.tile([128, N], F32, tag="ps", name="qk_ps")
            for k in range(KD):
                for h in range(2):
                    nc.tensor.matmul(ps[:, h * 512:(h + 1) * 512],
                                     wqkT[k][:, mc * 128:(mc + 1) * 128],
                                     xT[k][:, h * 512:(h + 1) * 512],
                                     start=(k == 0), stop=(k == KD - 1))
            nc.vector.tensor_copy(qkT[mc][:], ps[:])

        def emit_qkT_late(mc):
            ps = psumB.tile([128, N], F32, tag="pb", name="qk_ps2")
            for k in range(KD):
                for h in range(2):
                    nc.tensor.matmul(ps[:, h * 512:(h + 1) * 512],
                                     wqkT[k][:, mc * 128:(mc + 1) * 128],
                                     xT[k][:, h * 512:(h + 1) * 512],
                                     start=(k == 0), stop=(k == KD - 1))
            nc.vector.tensor_copy(qkT[mc][:], ps[:])

        for mc in (0, 6, 1, 7, 3, 9, 4, 10):  # j=0 routes: q {0,1,3,4}, k +6
            emit_qkT(mc)

        # ---- v[n, 192] + late qkT (fill PE slack under head-0's ACT-bound
        # dots phase)
        v_s = [const.tile([128, HPC * DH], BF16, tag=f"v{m}", name=f"v{m}")
               for m in range(NT)]

        def emit_v(mt):
            ps = psumB.tile([128, HPC * DH], F32, tag="pb", name="v_ps")
            for k in range(KD):
                nc.tensor.matmul(ps[:], xT[k][:, mt * 128:(mt + 1) * 128],
                                 wvT[k][:], start=(k == 0), stop=(k == KD - 1))
            nc.vector.tensor_copy(v_s[mt][:], ps[:])

        # ---- attention
        aoT = [const.tile([64, N], BF16, tag=f"aoT{j}", name=f"aoT{j}")
               for j in range(HPC)]

        def emit_av(j, attnT):
            av = psumB.tile([64, N], F32, tag="pb", name="av_ps")
            attnT4 = attnT[:].rearrange("p (a m q) -> p a m q", a=NT, m=NT)
            for mt in range(NT):
                for h in range(2):
                    nc.tensor.matmul(av[:, h * 512:(h + 1) * 512],
                                     v_s[mt][:, j * DH:(j + 1) * DH],
                                     attnT4[:, h * 4:(h + 1) * 4, mt, :],
                                     start=(mt == 0), stop=(mt == NT - 1))
            nc.vector.tensor_copy(aoT[j][:], av[:])
            # scatter aoT columns per destination rank: shard for global
            # rank s carries the columns of group-position s % GROUP (the
            # other batch's cores ignore it via zeroed woT rows)
            for dhalf in range(2):
                for s in range(GROUP):
                    r0 = dhalf * GROUP * DH + s * DH
                    nc.sync.dma_start(
                        a2ain[j][r0:r0 + DH, :],
                        aoT[j][:, s * NS:(s + 1) * NS])
            nc.gpsimd.collective_compute(
                "AllToAll", mybir.AluOpType.bypass, replica_groups=RG,
                ins=[a2ain[j][:, :].opt()], outs=[a2aout[j][:, :].opt()])

        attnT_prev = None
        for j in range(HPC):
            attnT = attp.tile([128, NT * N], BF16, tag="attnT", name="attnT")
            for qt in range(NT):
                acc = accp.tile([128, N], BF16, tag="acc", name="acc")
                s4 = sp.tile([128, ROUTES], F32, tag="s4", name="s4")
                r4 = sp.tile([128, ROUTES], F32, tag="r4", name="r4")
                es = []
                for r in range(ROUTES):
                    rj = r * HPC + j
                    tq, oq = divmod(rj * DH, 128)
                    dots = psumA.tile([128, N], F32, tag="ps", name="dots")
                    for h in range(2):
                        nc.tensor.matmul(
                            dots[:, h * 512:(h + 1) * 512],
                            qkT[tq][oq:oq + DH, qt * 128:(qt + 1) * 128],
                            qkT[6 + tq][oq:oq + DH, h * 512:(h + 1) * 512],
                            start=True, stop=True)
                    e = ep.tile([128, N], BF16, tag="e", name="e")
                    nc.scalar.activation(e[:], dots[:], Exp, scale=DH ** -0.5,
                                         accum_out=s4[:, r:r + 1])
                    es.append(e)
                nc.vector.reciprocal(r4[:], s4[:])
                nc.vector.tensor_scalar(acc[:], es[0][:], r4[:, 0:1], None,
                                        op0=mybir.AluOpType.mult)
                for r in range(1, ROUTES):
                    nc.vector.scalar_tensor_tensor(
                        acc[:], es[r][:], r4[:, r:r + 1], acc[:],
                        op0=mybir.AluOpType.mult, op1=mybir.AluOpType.max)
                # transpose acc [nq, m] -> attnT qt-block [m-local, mt, nq]
                dst = attnT[:, qt * N:(qt + 1) * N].rearrange(
                    "p (m q) -> p m q", m=NT)
                nc.sync.dma_start_transpose(dst, acc[:])
                # fill PE slack during head 0 with v and the late qkT tiles;
                # run the previous head's attn@v during heads 1 and 2
                if j == 0:
                    if qt < 4:
                        emit_qkT_late((2, 8, 5, 11)[qt])
                    else:
                        emit_v(qt - 4)
                        emit_v(qt)
                elif qt == 2:
                    emit_av(j - 1, attnT_prev)
            attnT_prev = attnT
        emit_av(HPC - 1, attnT_prev)

        # ---- projection for this core's 256 rows, over all 12 heads
        a2a_sb = [[const.tile([128, NS], BF16, tag=f"a2a{j}{kt}",
                              name=f"a2a{j}{kt}") for kt in range(4)]
                  for j in range(HPC)]
        for j in range(HPC):
            for kt in range(4):
                # gpsimd (SWDGE) queue: these wait on the collectives, and on
                # the sync HWDGE queue that wait would head-of-line block the
                # attention-phase DMA transposes behind it.
                nc.gpsimd.dma_start(a2a_sb[j][kt][:],
                                    a2aout[j][kt * 128:(kt + 1) * 128, :])
        JK = [(j, kt) for j in range(HPC) for kt in range(4)]
        for mchunk in range(2):
            ps = psumB.tile([128, D], F32, tag="pb", name="proj_ps")
            for c0, c1 in ((0, 512), (512, 768)):
                for i, (j, kt) in enumerate(JK):
                    nc.tensor.matmul(
                        ps[:, c0:c1],
                        a2a_sb[j][kt][:, mchunk * 128:(mchunk + 1) * 128],
                        woT[j][kt][:, c0:c1],
                        start=(i == 0), stop=(i == len(JK) - 1))
            po = pop.tile([128, D], F32, tag="po", name="po")
            nc.vector.tensor_add(po[:], ps[:], bias_s[:])
            nc.sync.dma_start(out_d[mchunk * 128:(mchunk + 1) * 128, :], po[:])

    nc.compile()
    return nc


def _prep_core_inputs(x, W_qkv, W_out, b_out, c):
    b, g = divmod(c, GROUP)
    bf = ml_dtypes.bfloat16
    xT = np.ascontiguousarray(x[b].T).astype(bf)
    wqkT = np.empty((D, CQK), dtype=bf)
    for r in range(ROUTES):
        for j in range(HPC):
            h = HPC * g + j
            rj = r * HPC + j
            wqkT[:, rj * DH:(rj + 1) * DH] = \
                W_qkv[r, h * DH:(h + 1) * DH, :].T.astype(bf)
            wqkT[:, INNER + rj * DH:INNER + (rj + 1) * DH] = \
                W_qkv[r, INNER + h * DH:INNER + (h + 1) * DH, :].T.astype(bf)
    wvT = np.empty((D, HPC * DH), dtype=bf)
    for j in range(HPC):
        h = HPC * g + j
        wvT[:, j * DH:(j + 1) * DH] = \
            W_qkv[0, 2 * INNER + h * DH:2 * INNER + (h + 1) * DH, :].T.astype(bf)
    # After the 8-rank per-head AllToAll, a2aout rows are (global source
    # rank s, dh t). Rows from the other batch's cores get zero weights.
    woT = np.zeros((HPC, NCORES * DH, D), dtype=bf)
    for j in range(HPC):
        for s in range(GROUP):
            sg = b * GROUP + s
            h = HPC * s + j
            woT[j, sg * DH:(sg + 1) * DH, :] = \
                W_out[:, h * DH:(h + 1) * DH].T.astype(bf)
    biasb = np.ascontiguousarray(
        np.broadcast_to(b_out.astype(np.float32), (128, D)))
    return {"xT": xT, "wqkT": wqkT, "wvT": wvT, "woT": woT, "biasb": biasb}


def _run(in_maps, trace=False, tmpdir=None):
    if "nc" not in _CACHE:
        _CACHE["nc"] = _build_nc()
    return bass_utils.run_bass_kernel_spmd(
        _CACHE["nc"], in_maps, core_ids=list(range(NCORES)),
        trace=trace, tmpdir=tmpdir)


def kernel(x, W_qkv, W_out, b_out, heads, _trace=False, _tmpdir=None):
    x = np.asarray(x, dtype=np.float32)
    W_qkv = np.asarray(W_qkv, dtype=np.float32)
    W_out = np.asarray(W_out, dtype=np.float32)
    b_out = np.asarray(b_out, dtype=np.float32)
    assert int(heads) == HEADS
    in_maps = [_prep_core_inputs(x, W_qkv, W_out, b_out, c)
               for c in range(NCORES)]
    res = _run(in_maps, trace=_trace, tmpdir=_tmpdir)
    out = np.empty((B, N, D), dtype=np.float32)
    for c in range(NCORES):
        b, g = divmod(c, GROUP)
        out[b, g * 256:(g + 1) * 256, :] = res.results[c]["out"]
    if _trace:
        return out, res
    return out
